# revision 1
# baseline (speedup 1.0000x reference)
"""Trainium2 Bass kernel for nn_CRA_46797963657479.

Math: the reference builds per-batch gram matrix A = cat_phi^T cat_phi
([B,392,392]) and feeds concat(A, A^T) through big 1x1 convs.  Since A is
symmetric and everything after cat_phi is linear, the whole tail collapses:

    W[b, l] = (u3 + cat_phi[b] @ u4) . cat_phi[b][:, l] + K
    out[b]  = xp[b] * W[b, :N] + yp[b] * W[b, N:]

with u3 = w5a @ w3, u4 = w5b @ (w4[:, :2N] + w4[:, 2N:]),
K = w5a.b3 + w5b.b4 + b5.  BN folds into the conv weights.  What remains per
batch is two 192x192 matmuls (phi_x, phi_y), a weighted free-dim reduction
(z), one more matmul for W, and an elementwise combine -> memory-bound.

Sharding: pure data parallel, batch 256 -> 32 per core on 8 cores.
"""

import os
import ml_dtypes
import numpy as np

import concourse.bass as bass
import concourse.bacc as bacc
import concourse.tile as tile
from concourse import mybir
from concourse.bass_utils import run_bass_kernel_spmd

F32 = mybir.dt.float32
F32R = mybir.dt.float32r
BF16 = mybir.dt.bfloat16

B, N, C = 256, 196, 192
NCORES = 8
NB = B // NCORES          # 32 batches per core
NPAIR = NB // 2           # 16 pairs per core
L = 2 * N                 # 392 free columns per pair tile / per stream-pack
CLO, CHI = 128, C - 128   # 128 + 64 channel split
CHIA = CHI + 1            # hi chunk augmented with a ones-row (folds +K)
CB_COLS = 1562            # const blob columns

_CACHE = {}


def _build_program(sim_safe=False):
    nc = bacc.Bacc("TRN2", target_bir_lowering=False, debug=False)

    xy = nc.dram_tensor("xy", [NB, C, 2, N], F32R, kind="ExternalInput")
    out = nc.dram_tensor("out", [NB, C, N], F32, kind="ExternalOutput")
    # bf16 copy of xy for the phi matmuls (final combine reads the f32 xy)
    xyb = nc.dram_tensor("xyb", [NB, C, 2, N], BF16, kind="ExternalInput")
    # all constants packed into one blob: [128 partitions, 1562 f32 cols]
    cblob = nc.dram_tensor("cblob", [CLO, CB_COLS], F32R, kind="ExternalInput")
    # bf16 weight blob: 8 lhsT tiles packed along free dim
    wblob = nc.dram_tensor("wblob", [CLO, 772], BF16, kind="ExternalInput")

    xyc = xy.rearrange("b c s n -> c b s n")     # [C, NB, 2, N]
    xybc = xyb.rearrange("b c s n -> c b s n")
    outv = out.rearrange("b c n -> c b n")       # [C, NB, N]

    with tile.TileContext(nc) as tc:
        with (
            tc.tile_pool(name="consts", bufs=1) as consts,
            tc.tile_pool(name="xin", bufs=4) as xin,
            tc.tile_pool(name="phi", bufs=4) as phip,
            tc.tile_pool(name="junk", bufs=3) as junkp,
            tc.tile_pool(name="qp", bufs=4) as qp,
            tc.tile_pool(name="work", bufs=3) as work,
            tc.tile_pool(name="outp", bufs=3) as outp,
            tc.tile_pool(name="psph", bufs=1, space="PSUM") as psph,
            tc.tile_pool(name="psw", bufs=1, space="PSUM") as psw,
        ):
            wb = consts.tile([CLO, 772], BF16)
            nc.sync.dma_start(out=wb[:], in_=wblob[:])
            blob = consts.tile([CLO, CB_COLS], F32R)

            def bv(c0, ncols, rows=CLO, dt=F32):
                ap = blob[0:rows, c0:c0 + ncols]
                return ap if dt is F32R else ap.bitcast(dt)

            twxa = wb[:, 0:128]
            twxb = wb[:, 128:193]
            twxc = wb[0:CHI, 193:321]
            twxd = wb[0:CHI, 321:386]
            twya = wb[:, 386:514]
            twyb = wb[:, 514:579]
            twyc = wb[0:CHI, 579:707]
            twyd = wb[0:CHI, 707:772]
            tc1lo = bv(772, 1)
            tc1hi = bv(773, 1, rows=CHIA)
            tc2lo = bv(774, 1)
            tc2hi = bv(775, 1, rows=CHIA)
            tu4lo = bv(776, L)
            tu4hi = bv(1168, L, rows=CHIA)
            tu3lo = bv(1560, 1)
            tu3hi = bv(1561, 1, rows=CHIA)

            def f(ap):
                return ap.bitcast(F32)

            relu = mybir.ActivationFunctionType.Relu
            mult = mybir.AluOpType.mult
            add = mybir.AluOpType.add
            byp = mybir.AluOpType.bypass

            GP = 2  # pairs per I/O group

            def emit_front(g, u, xg, xgb, og):  # og is (og, og2)
                """loads are per group; phi matmuls, relu, z, q for pair."""
                b0 = 2 * u
                xlo2 = xgb[:, 0, b0:b0 + 2, 0, :]
                xhi2 = xgb[0:CHI, 1, b0:b0 + 2, 0, :]
                ylo2 = xgb[:, 0, b0:b0 + 2, 1, :]
                yhi2 = xgb[0:CHI, 1, b0:b0 + 2, 1, :]

                par = (GP * g + u) % 2
                ps_xlo = psph.tile([CLO, L], F32, tag=f"ps_xlo{par}")
                ps_xhi = psph.tile([CHIA, L], F32)
                ps_ylo = psph.tile([CLO, L], F32, tag=f"ps_ylo{par}")
                ps_yhi = psph.tile([CHIA, L], F32)
                nc.tensor.matmul(ps_xlo[:], twxa, xlo2, start=True, stop=False)
                nc.tensor.matmul(ps_xlo[:], twxc, xhi2, start=False, stop=True)
                nc.tensor.matmul(ps_xhi[:], twxb, xlo2, start=True, stop=False)
                nc.tensor.matmul(ps_xhi[:], twxd, xhi2, start=False, stop=True)
                nc.tensor.matmul(ps_ylo[:], twya, ylo2, start=True, stop=False)
                nc.tensor.matmul(ps_ylo[:], twyc, yhi2, start=False, stop=True)
                nc.tensor.matmul(ps_yhi[:], twyb, ylo2, start=True, stop=False)
                nc.tensor.matmul(ps_yhi[:], twyd, yhi2, start=False, stop=True)

                phiA_lo = phip.tile([CLO, L], BF16)
                phiB_lo = phip.tile([CLO, L], BF16)
                phiA_hi = phip.tile([CHIA, L], BF16)
                phiB_hi = phip.tile([CHIA, L], BF16)
                nc.scalar.activation(phiA_lo[:, 0:N], ps_xlo[:, 0:N], relu, bias=tc1lo)
                nc.scalar.activation(phiA_lo[:, N:L], ps_ylo[:, 0:N], relu, bias=tc2lo)
                nc.scalar.activation(phiB_lo[:, 0:N], ps_xlo[:, N:L], relu, bias=tc1lo)
                nc.scalar.activation(phiB_lo[:, N:L], ps_ylo[:, N:L], relu, bias=tc2lo)
                nc.scalar.activation(phiA_hi[:, 0:N], ps_xhi[:, 0:N], relu, bias=tc1hi)
                nc.scalar.activation(phiA_hi[:, N:L], ps_yhi[:, 0:N], relu, bias=tc2hi)
                nc.scalar.activation(phiB_hi[:, 0:N], ps_xhi[:, N:L], relu, bias=tc1hi)
                nc.scalar.activation(phiB_hi[:, N:L], ps_yhi[:, N:L], relu, bias=tc2hi)

                jA_lo = junkp.tile([CLO, L], F32, tag="j_lo")
                jB_lo = junkp.tile([CLO, L], F32, tag="j_lo")
                jA_hi = junkp.tile([CHIA, L], F32, tag="j_hi")
                jB_hi = junkp.tile([CHIA, L], F32, tag="j_hi")
                zA_lo = qp.tile([CLO, 1], F32)
                zB_lo = qp.tile([CLO, 1], F32)
                zA_hi = qp.tile([CHIA, 1], F32)
                zB_hi = qp.tile([CHIA, 1], F32)
                nc.vector.scalar_tensor_tensor(
                    out=jA_lo[:], in0=phiA_lo[:], scalar=1.0, in1=tu4lo,
                    op0=byp, op1=mult, accum_out=zA_lo[:])
                nc.vector.scalar_tensor_tensor(
                    out=jB_lo[:], in0=phiB_lo[:], scalar=1.0, in1=tu4lo,
                    op0=byp, op1=mult, accum_out=zB_lo[:])
                nc.vector.scalar_tensor_tensor(
                    out=jA_hi[:], in0=phiA_hi[:], scalar=1.0, in1=tu4hi,
                    op0=byp, op1=mult, accum_out=zA_hi[:])
                nc.vector.scalar_tensor_tensor(
                    out=jB_hi[:], in0=phiB_hi[:], scalar=1.0, in1=tu4hi,
                    op0=byp, op1=mult, accum_out=zB_hi[:])

                qA_lo = qp.tile([CLO, 1], BF16, tag="qb_lo")
                qB_lo = qp.tile([CLO, 1], BF16, tag="qb_lo")
                qA_hi = qp.tile([CHIA, 1], BF16, tag="qb_hi")
                qB_hi = qp.tile([CHIA, 1], BF16, tag="qb_hi")
                nc.gpsimd.tensor_scalar(qA_lo[:], zA_lo[:], tu3lo, None, add)
                nc.gpsimd.tensor_scalar(qB_lo[:], zB_lo[:], tu3lo, None, add)
                nc.gpsimd.tensor_scalar(qA_hi[:], zA_hi[:], tu3hi, None, add)
                nc.gpsimd.tensor_scalar(qB_hi[:], zB_hi[:], tu3hi, None, add)
                return dict(g=g, u=u, xg=xg, og=og,
                            phi=(phiA_lo, phiA_hi, phiB_lo, phiB_hi),
                            q=(qA_lo, qA_hi, qB_lo, qB_hi))

            def emit_back(st):
                g, u, xg = st["g"], st["u"], st["xg"]
                phiA_lo, phiA_hi, phiB_lo, phiB_hi = st["phi"]
                qA_lo, qA_hi, qB_lo, qB_hi = st["q"]
                b0 = 2 * u
                ps_w = psw.tile([CLO, 2, 512], F32, tag="ps_w")
                nc.tensor.matmul(ps_w[:, 0, 0:L], qA_lo[:].broadcast_to([CLO, CLO]),
                                 phiA_lo[:], start=True, stop=False)
                nc.tensor.matmul(ps_w[:, 0, 0:L], qA_hi[:].broadcast_to([CHIA, CLO]),
                                 phiA_hi[:], start=False, stop=True)
                nc.tensor.matmul(ps_w[:, 1, 0:L], qB_lo[:].broadcast_to([CLO, CLO]),
                                 phiB_lo[:], start=True, stop=False)
                nc.tensor.matmul(ps_w[:, 1, 0:L], qB_hi[:].broadcast_to([CHIA, CLO]),
                                 phiB_hi[:], start=False, stop=True)

                wx = ps_w[:, :, 0:N]
                wxb2 = bass.AP(tensor=wx.tensor, offset=wx.offset,
                               ap=[wx.ap[0], [0, 2]] + list(wx.ap[1:]))
                wy = ps_w[:, :, N:L]
                wyb2 = bass.AP(tensor=wy.tensor, offset=wy.offset,
                               ap=[wy.ap[0], [0, 2]] + list(wy.ap[1:]))
                og, _og2 = st["og"]
                t1 = work.tile([CLO, 2, 2, N], F32, tag="t1")
                t2 = work.tile([CLO, 2, 2, N], F32, tag="t2")
                nc.vector.tensor_mul(t1[:], f(xg[:, :, b0:b0 + 2, 0, :]), wxb2)
                nc.vector.tensor_mul(t2[:], f(xg[:, :, b0:b0 + 2, 1, :]), wyb2)
                nc.vector.tensor_add(og[:, :, b0:b0 + 2, :], t1[:], t2[:])
                if u == GP - 1:
                    gb = 2 * GP * g
                    nc.scalar.dma_start(out=outv[0:CLO, gb:gb + 2 * GP, :], in_=og[:, 0, :, :])
                    nc.scalar.dma_start(out=outv[CLO:C, gb:gb + 2 * GP, :], in_=og[0:CHI, 1, :, :])

            LAG = 2
            # PE HAM warm-up: keep the array busy from t~0.5us so the
            # clock promotes to 2.4GHz before the first real matmul.
            wseed = consts.tile([CLO, 640], BF16)
            nc.vector.memset(wseed[:], 1.0)
            wup = psw.tile([CLO, 2, 512], F32, tag="ps_w")
            for _ in range(24):
                nc.tensor.matmul(wup[:, 0, :], wseed[:, 512:640],
                                 wseed[:, 0:512], start=True, stop=True)

            pending = []
            for g in range(NPAIR // GP):
                gb = 2 * GP * g
                xgb = xin.tile([CLO, 2, 2 * GP, 2, N], BF16, tag="xgb")
                nc.sync.dma_start(out=xgb[:, 0, :, :, :], in_=xybc[0:CLO, gb:gb + 2 * GP, :, :])
                nc.sync.dma_start(out=xgb[0:CHI, 1, :, :, :], in_=xybc[CLO:C, gb:gb + 2 * GP, :, :])
                xg = xin.tile([CLO, 2, 2 * GP, 2, N], F32R)
                if sim_safe:
                    nc.gpsimd.memset(xg[CHI:CLO, 1, :, :, :].bitcast(F32), 0.0)
                nc.scalar.dma_start(out=xg[:, 0, :, :, :], in_=xyc[0:CLO, gb:gb + 2 * GP, :, :])
                nc.scalar.dma_start(out=xg[0:CHI, 1, :, :, :], in_=xyc[CLO:C, gb:gb + 2 * GP, :, :])
                if g == 0:
                    nc.sync.dma_start(out=blob[:], in_=cblob[:])
                og = outp.tile([CLO, 2, 2 * GP, N], F32, tag="og")
                for u in range(GP):
                    pending.append(emit_front(g, u, xg, xgb, (og, None)))
                    if len(pending) > LAG:
                        emit_back(pending.pop(0))
            for st in pending:
                emit_back(st)

    nc.compile()
    return nc


def _host_prepack(d):
    """Fold BN, collapse the linear tail, build per-core constant arrays."""
    f = np.float32
    inv1 = d["g1"] / np.sqrt(d["v1"] + 1e-5)
    W1 = (d["w1"] * inv1[:, None]).astype(f)
    c1 = ((d["b1"] - d["m1"]) * inv1 + d["be1"]).astype(f)
    inv2 = d["g2"] / np.sqrt(d["v2"] + 1e-5)
    W2 = (d["w2"] * inv2[:, None]).astype(f)
    c2 = ((d["b2"] - d["m2"]) * inv2 + d["be2"]).astype(f)

    w4eff = d["w4"][:, :L] + d["w4"][:, L:]
    w5a, w5b = d["w5"][0, :C], d["w5"][0, C:]
    u3 = (w5a @ d["w3"]).astype(f)
    u4 = (w5b @ w4eff).astype(f)
    K = float(w5a @ d["b3"] + w5b @ d["b4"] + d["b5"][0])

    W1T, W2T = np.ascontiguousarray(W1.T), np.ascontiguousarray(W2.T)

    def hi_pad_m(a):  # [k, 64] -> [k, 65] with zero last col
        z = np.zeros((a.shape[0], CHIA), f)
        z[:, :CHI] = a
        return z

    blob = np.zeros((CLO, CB_COLS), f)

    def put(c0, arr, rows=None):
        a = np.asarray(arr, f)
        r = a.shape[0]
        blob[:r, c0:c0 + a.shape[1]] = a

    put(772, c1[:CLO, None])
    put(773, np.concatenate([c1[CLO:C], [f(1.0)]])[:, None])
    put(774, c2[:CLO, None])
    put(775, np.concatenate([c2[CLO:C], [f(1.0)]])[:, None])
    put(776, np.broadcast_to(u4, (CLO, L)))
    put(1168, np.concatenate([np.broadcast_to(u4, (CHI, L)),
                              np.zeros((1, L), f)], axis=0))
    put(1560, u3[:CLO, None])
    put(1561, np.concatenate([u3[CLO:C], [f(K)]])[:, None])
    wblob = np.zeros((CLO, 772), ml_dtypes.bfloat16)

    def wput(c0, arr):
        a = np.asarray(arr, f)
        wblob[:a.shape[0], c0:c0 + a.shape[1]] = a.astype(ml_dtypes.bfloat16)

    wput(0, W1T[:CLO, :CLO])
    wput(128, hi_pad_m(W1T[:CLO, CLO:C]))
    wput(193, W1T[CLO:C, :CLO])
    wput(321, hi_pad_m(W1T[CLO:C, CLO:C]))
    wput(386, W2T[:CLO, :CLO])
    wput(514, hi_pad_m(W2T[:CLO, CLO:C]))
    wput(579, W2T[CLO:C, :CLO])
    wput(707, hi_pad_m(W2T[CLO:C, CLO:C]))
    return {"cblob": blob, "wblob": wblob}


def run(inputs, trace=False):
    d = {k: np.asarray(v) for k, v in inputs.items()}
    consts = _host_prepack(d)

    xyp = np.empty((B, C, 2, N), np.float32)
    xyp[:, :, 0] = d["x"].transpose(0, 2, 1)
    xyp[:, :, 1] = d["y"].transpose(0, 2, 1)

    if "nc" not in _CACHE:
        _CACHE["nc"] = _build_program()
    nc = _CACHE["nc"]

    in_maps = []
    xybp = xyp.astype(ml_dtypes.bfloat16)
    for cid in range(NCORES):
        m = dict(consts)
        m["xy"] = np.ascontiguousarray(xyp[cid * NB:(cid + 1) * NB])
        m["xyb"] = np.ascontiguousarray(xybp[cid * NB:(cid + 1) * NB])
        in_maps.append(m)

    res = run_bass_kernel_spmd(nc, in_maps, list(range(NCORES)), trace=trace)
    out = np.concatenate([res.results[i]["out"] for i in range(NCORES)], axis=0)
    return out, res


def kernel(**inputs):
    out, _ = run(inputs, trace=False)
    return out



# revision 7
# speedup vs baseline: 1.0441x; 1.0441x over previous
"""Trainium2 Bass kernel for nn_CRA_46797963657479.

Math: the tail after cat_phi is linear in the gram matrix A, so it collapses:
    q[b]   = u3 + cat_phi[b] @ u4            (per-channel scalar, C)
    W[b,l] = q[b] . cat_phi[b][:,l] + K
    out[b] = xp[b] * W[b,:N] + yp[b] * W[b,N:]
with u3 = w3^T w5a, u4 = (w4[:, :2N]+w4[:, 2N:])^T w5b,
K = w5a.b3 + w5b.b4 + b5; BN folds into conv weights (W1',c1 / W2',c2).

Device pipeline per batch-pair (2 batches):
  PE   : 12 front matmuls (phi pre-act, bias via ones-row in x-hi chunk)
         + 4 back matmuls (q broadcast -> W on all partitions)
  DVE  : z = sum_l relu(ps)*u4 via scalar_tensor_tensor(max0, mult, accum)
         straight from PSUM (relu folded, no dependency on ACT),
         q = z + u3 (one tiny [128,3] add), y-side combine mult + adds
  ACT  : phi = relu(ps) -> SBUF bf16 (pure relu, no bias), 2 ops/pair
  GPS  : x-side combine mults (stt from PSUM), output DMA triggers (SWDGE)
  DMA  : bf16-only I/O; inputs on sync HWDGE queue, outputs on gpsimd queue

Channels 192 = 128 (lo) + 64 (hi); hi chunks of PSUM/phi are batch-packed
on 128 partitions (A on 0:64, B on 64:128). Output is written bf16 and
upcast to f32 on the host. Sharding: pure data parallel, 32 batches/core.
"""

import ml_dtypes
import numpy as np

import concourse.bass as bass
import concourse.bacc as bacc
import concourse.tile as tile
from concourse import mybir
from concourse.bass_utils import run_bass_kernel_spmd

F32 = mybir.dt.float32
BF16 = mybir.dt.bfloat16

B, N, C = 256, 196, 192
NCORES = 8
NB = B // NCORES          # 32 batches per core
NPAIR = NB // 2           # 16 pairs per core
L = 2 * N                 # 392 cat columns per batch
CLO, CHI = 128, C - 128   # 128 + 64 channel split
CHIA = CHI + 1            # hi input chunk + ones row (folds conv/BN bias)
SG = 2                    # pairs per input-DMA supergroup
OSG = 4                   # pairs per output-DMA supergroup
NSG = NPAIR // SG
NOSG = NPAIR // OSG

_CACHE = {}


def _build_program(gps_stt=False, out_eng="gpsimd"):
    nc = bacc.Bacc("TRN2", target_bir_lowering=False, debug=False)

    xlo_d = nc.dram_tensor("xlo", [NSG, CLO, SG * 2, 2, N], BF16, kind="ExternalInput")
    xhi_d = nc.dram_tensor("xhi", [NSG, CHIA, SG * 2, 2, N], BF16, kind="ExternalInput")
    wblob = nc.dram_tensor("wblob", [CLO, 768], BF16, kind="ExternalInput")
    cblob = nc.dram_tensor("cblob", [CLO, 398], F32, kind="ExternalInput")
    olo_d = nc.dram_tensor("olo", [NOSG, CLO, OSG * 2, N], BF16, kind="ExternalOutput")
    ohi_d = nc.dram_tensor("ohi", [NOSG, CHI, OSG * 2, N], BF16, kind="ExternalOutput")

    relu = mybir.ActivationFunctionType.Relu
    mult = mybir.AluOpType.mult
    add = mybir.AluOpType.add
    amax = mybir.AluOpType.max

    with tile.TileContext(nc) as tc:
        with (
            tc.tile_pool(name="consts", bufs=1) as consts,
            tc.tile_pool(name="xin", bufs=2) as xin,
            tc.tile_pool(name="phip", bufs=3) as phip,
            tc.tile_pool(name="junkp", bufs=2) as junkp,
            tc.tile_pool(name="zqp", bufs=3) as zqp,
            tc.tile_pool(name="workp", bufs=2) as workp,
            tc.tile_pool(name="outp", bufs=2) as outp,
            tc.tile_pool(name="pslo", bufs=2, space="PSUM") as pslo,
            tc.tile_pool(name="pshi", bufs=1, space="PSUM") as pshi,
            tc.tile_pool(name="psw", bufs=1, space="PSUM") as psw,
        ):
            wb = consts.tile([CLO, 768], BF16)
            nc.scalar.dma_start(out=wb[:], in_=wblob[:])
            cb = consts.tile([CLO, 398], F32)
            nc.scalar.dma_start(out=cb[:], in_=cblob[:])

            # weight tiles (lhsT = [cin, cout]); hi-contract tiles carry a
            # 65th row with the folded conv+BN bias
            twxa = wb[:, 0:128]
            twxb = wb[:, 128:192]
            twya = wb[:, 192:320]
            twyb = wb[:, 320:384]
            twxc = wb[0:CHIA, 384:512]
            twxd = wb[0:CHIA, 512:576]
            twyc = wb[0:CHIA, 576:704]
            twyd = wb[0:CHIA, 704:768]

            # u4 broadcast on all partitions f32 [128, 392] (+ 3 u3 cols)
            tu4f = cb[:, 0:392]
            tu3 = cb[:, 392:395]
            # [128, 2, 196] strided view of u4 (matches phi A/B slice shapes)
            tu4v = bass.AP(tensor=tu4f.tensor, offset=tu4f.offset,
                           ap=[tu4f.ap[0], [N, 2], [1, N]])

            pairs = []

            def emit_front(p, xl, xh, j):
                """j = pair index within its input supergroup."""
                xlo = xl[:, 2 * j:2 * j + 2, :, :]      # [128, 2b, 2s, 196]
                xhi = xh[:, 2 * j:2 * j + 2, :, :]      # [65, 2b, 2s, 196]

                ps_lo = pslo.tile([CLO, 2, 512], F32)   # banks: 0=x, 1=y
                ps_hi = pshi.tile([CLO, 512], F32)      # A on 0:64, B on 64:128

                nc.tensor.matmul(ps_lo[:, 0, 0:L], twxa, xlo[:, :, 0, :],
                                 start=True, stop=False)
                nc.tensor.matmul(ps_lo[:, 0, 0:L], twxc, xhi[:, :, 0, :],
                                 start=False, stop=True)
                nc.tensor.matmul(ps_lo[:, 1, 0:L], twya, xlo[:, :, 1, :],
                                 start=True, stop=False)
                nc.tensor.matmul(ps_lo[:, 1, 0:L], twyc, xhi[:, :, 1, :],
                                 start=False, stop=True)
                for b in (0, 1):
                    pb = slice(64 * b, 64 * b + 64)
                    nc.tensor.matmul(ps_hi[pb, 0:N], twxb, xlo[:, b, 0, :],
                                     start=True, stop=False)
                    nc.tensor.matmul(ps_hi[pb, 0:N], twxd, xhi[:, b, 0, :],
                                     start=False, stop=True)
                    nc.tensor.matmul(ps_hi[pb, N:L], twyb, xlo[:, b, 1, :],
                                     start=True, stop=False)
                    nc.tensor.matmul(ps_hi[pb, N:L], twyd, xhi[:, b, 1, :],
                                     start=False, stop=True)

                # phi -> SBUF bf16 (pure relu; bias already in PSUM)
                phi_lo = phip.tile([CLO, 2, L], BF16, tag="phi_lo")
                phi_hi = phip.tile([CLO, L], BF16, tag="phi_hi")
                nc.scalar.activation(phi_lo[:], ps_lo[:, 0:2, 0:L], relu)
                nc.scalar.activation(phi_hi[:], ps_hi[:, 0:L], relu)

                # z = sum_l relu(ps)*u4 straight from PSUM (relu folded)
                zf = zqp.tile([CLO, 3], F32, tag="zf")
                jA = junkp.tile([CLO, 2, N], BF16, tag="jA")
                jB = junkp.tile([CLO, 2, N], BF16, tag="jB")
                jh = junkp.tile([CLO, L], BF16, tag="jh")
                nc.vector.scalar_tensor_tensor(
                    out=jA[:], in0=ps_lo[:, 0:2, 0:N], scalar=0.0, in1=tu4v,
                    op0=amax, op1=mult, accum_out=zf[:, 0:1])
                nc.vector.scalar_tensor_tensor(
                    out=jB[:], in0=ps_lo[:, 0:2, N:L], scalar=0.0, in1=tu4v,
                    op0=amax, op1=mult, accum_out=zf[:, 1:2])
                nc.vector.scalar_tensor_tensor(
                    out=jh[:], in0=ps_hi[:, 0:L], scalar=0.0, in1=tu4f,
                    op0=amax, op1=mult, accum_out=zf[:, 2:3])

                # q = z + u3, bf16 (cols: A-lo, B-lo, hi batch-packed)
                qb = zqp.tile([CLO, 3], BF16, tag="qb")
                nc.vector.scalar_tensor_tensor(
                    out=qb[:], in0=zf[:], scalar=0.0, in1=tu3,
                    op0=mybir.AluOpType.bypass, op1=add)
                return dict(p=p, xlo=xlo, xhi=xhi,
                            phi_lo=phi_lo, phi_hi=phi_hi, qb=qb)

            def emit_back(st, og, jo):
                xlo, xhi = st["xlo"], st["xhi"]
                phi_lo, phi_hi, qb = st["phi_lo"], st["phi_hi"], st["qb"]

                ps_w = psw.tile([CLO, 2, 512], F32)     # banks: 0=W_A, 1=W_B
                nc.tensor.matmul(ps_w[:, 0, 0:L],
                                 qb[:, 0:1].broadcast_to([CLO, CLO]),
                                 phi_lo[:, 0:2, 0:N], start=True, stop=False)
                nc.tensor.matmul(ps_w[:, 0, 0:L],
                                 qb[0:64, 2:3].broadcast_to([64, CLO]),
                                 phi_hi[0:64, :], start=False, stop=True)
                nc.tensor.matmul(ps_w[:, 1, 0:L],
                                 qb[:, 1:2].broadcast_to([CLO, CLO]),
                                 phi_lo[:, 0:2, N:L], start=True, stop=False)
                nc.tensor.matmul(ps_w[:, 1, 0:L],
                                 qb[64:128, 2:3].broadcast_to([64, CLO]),
                                 phi_hi[64:128, :], start=False, stop=True)

                # combine: og = x*(Wx+K) + y*(Wy+K); banks of ps_w = batch
                t1 = workp.tile([CLO, 2, 2, N], BF16, tag="t1")
                t2 = workp.tile([CLO, 2, 2, N], BF16, tag="t2")
                # x-side on gpsimd (falls back to vector if gps_stt=False)
                e1 = nc.gpsimd if gps_stt else nc.vector
                e1.scalar_tensor_tensor(
                    out=t1[:, 0, :, :], in0=ps_w[:, 0:2, 0:N], scalar=KHOLD,
                    op0=add, op1=mult, in1=xlo[:, :, 0, :])
                e1.scalar_tensor_tensor(
                    out=t1[0:CHI, 1, :, :], in0=ps_w[0:CHI, 0:2, 0:N],
                    scalar=KHOLD, op0=add, op1=mult, in1=xhi[0:CHI, :, 0, :])
                # y-side on vector
                nc.vector.scalar_tensor_tensor(
                    out=t2[:, 0, :, :], in0=ps_w[:, 0:2, N:L], scalar=KHOLD,
                    op0=add, op1=mult, in1=xlo[:, :, 1, :])
                nc.vector.scalar_tensor_tensor(
                    out=t2[0:CHI, 1, :, :], in0=ps_w[0:CHI, 0:2, N:L],
                    scalar=KHOLD, op0=add, op1=mult, in1=xhi[0:CHI, :, 1, :])
                b0 = 2 * jo
                nc.vector.tensor_add(og[:, 0, b0:b0 + 2, :],
                                     t1[:, 0, :, :], t2[:, 0, :, :])
                nc.vector.tensor_add(og[0:CHI, 1, b0:b0 + 2, :],
                                     t1[0:CHI, 1, :, :], t2[0:CHI, 1, :, :])

            oeng = {"gpsimd": nc.gpsimd, "scalar": nc.scalar,
                    "sync": nc.sync}[out_eng]
            og = None
            for p in range(NPAIR):
                if p % SG == 0:
                    g = p // SG
                    xl = xin.tile([CLO, SG * 2, 2, N], BF16, tag="xl")
                    xh = xin.tile([CHIA, SG * 2, 2, N], BF16, tag="xh")
                    nc.sync.dma_start(out=xl[:], in_=xlo_d[g])
                    nc.sync.dma_start(out=xh[:], in_=xhi_d[g])
                if p % OSG == 0:
                    og = outp.tile([CLO, 2, OSG * 2, N], BF16, tag="og")
                st = emit_front(p, xl, xh, p % SG)
                pairs.append((st, og, p % OSG))
                if len(pairs) > 1:
                    sst, sog, sjo = pairs.pop(0)
                    emit_back(sst, sog, sjo)
                    if sjo == OSG - 1:
                        go = sst["p"] // OSG
                        oeng.dma_start(out=olo_d[go], in_=sog[:, 0, :, :])
                        oeng.dma_start(out=ohi_d[go], in_=sog[0:CHI, 1, :, :])
            while pairs:
                sst, sog, sjo = pairs.pop(0)
                emit_back(sst, sog, sjo)
                if sjo == OSG - 1:
                    go = sst["p"] // OSG
                    oeng.dma_start(out=olo_d[go], in_=sog[:, 0, :, :])
                    oeng.dma_start(out=ohi_d[go], in_=sog[0:CHI, 1, :, :])

    nc.compile()
    return nc


KHOLD = 0.0  # patched per-run before program build (K is a host constant)


def _host_prepack(d):
    """Fold BN, collapse the linear tail, build per-core constant arrays."""
    f = np.float32
    bf = ml_dtypes.bfloat16
    inv1 = d["g1"] / np.sqrt(d["v1"] + 1e-5)
    W1 = (d["w1"] * inv1[:, None]).astype(f)
    c1 = ((d["b1"] - d["m1"]) * inv1 + d["be1"]).astype(f)
    inv2 = d["g2"] / np.sqrt(d["v2"] + 1e-5)
    W2 = (d["w2"] * inv2[:, None]).astype(f)
    c2 = ((d["b2"] - d["m2"]) * inv2 + d["be2"]).astype(f)

    w4eff = d["w4"][:, :L] + d["w4"][:, L:]
    w5a, w5b = d["w5"][0, :C], d["w5"][0, C:]
    u3 = (w5a @ d["w3"]).astype(f)
    u4 = (w5b @ w4eff).astype(f)
    K = float(w5a @ d["b3"] + w5b @ d["b4"] + d["b5"][0])

    W1T = np.ascontiguousarray(W1.T)  # [cin, cout]
    W2T = np.ascontiguousarray(W2.T)

    wblob = np.zeros((CLO, 768), bf)

    def wput(c0, arr):
        a = np.asarray(arr, f)
        wblob[:a.shape[0], c0:c0 + a.shape[1]] = a.astype(bf)

    wput(0, W1T[:CLO, :CLO])
    wput(128, W1T[:CLO, CLO:C])
    wput(192, W2T[:CLO, :CLO])
    wput(320, W2T[:CLO, CLO:C])
    wput(384, np.concatenate([W1T[CLO:C, :CLO], c1[None, :CLO]], axis=0))
    wput(512, np.concatenate([W1T[CLO:C, CLO:C], c1[None, CLO:C]], axis=0))
    wput(576, np.concatenate([W2T[CLO:C, :CLO], c2[None, :CLO]], axis=0))
    wput(704, np.concatenate([W2T[CLO:C, CLO:C], c2[None, CLO:C]], axis=0))

    cblob = np.zeros((CLO, 398), f)
    cblob[:, 0:392] = np.broadcast_to(u4, (CLO, L))
    cblob[:, 392] = u3[:CLO]
    cblob[:, 393] = u3[:CLO]
    hi2 = np.concatenate([u3[CLO:C], u3[CLO:C]])
    cblob[:, 394] = hi2
    return {"wblob": wblob, "cblob": cblob}, K


def _pack_inputs(d):
    bf = ml_dtypes.bfloat16
    xt = np.ascontiguousarray(d["x"].transpose(0, 2, 1))  # [B, C, N]
    yt = np.ascontiguousarray(d["y"].transpose(0, 2, 1))
    xy = np.stack([xt, yt], axis=2).astype(bf)            # [B, C, 2, N]
    # [B, C, 2, N] -> per core [NSG, part, SG*2, 2, N]
    xlo = np.zeros((NCORES, NSG, CLO, SG * 2, 2, N), bf)
    xhi = np.zeros((NCORES, NSG, CHIA, SG * 2, 2, N), bf)
    v = xy.reshape(NCORES, NSG, SG * 2, C, 2, N)
    xlo[:] = v[:, :, :, 0:CLO].transpose(0, 1, 3, 2, 4, 5)
    xhi[:, :, 0:CHI] = v[:, :, :, CLO:C].transpose(0, 1, 3, 2, 4, 5)
    xhi[:, :, CHI] = 1.0
    return xlo, xhi


def run(inputs, trace=False):
    global KHOLD
    d = {k: np.asarray(v) for k, v in inputs.items()}
    consts, K = _host_prepack(d)
    KHOLD = K

    if "nc" not in _CACHE:
        _CACHE["nc"] = _build_program()
    nc = _CACHE["nc"]

    xlo, xhi = _pack_inputs(d)
    in_maps = []
    for cid in range(NCORES):
        m = dict(consts)
        m["xlo"] = np.ascontiguousarray(xlo[cid])
        m["xhi"] = np.ascontiguousarray(xhi[cid])
        in_maps.append(m)

    res = run_bass_kernel_spmd(nc, in_maps, list(range(NCORES)), trace=trace)

    out = np.empty((B, C, N), np.float32)
    for cid in range(NCORES):
        olo = np.asarray(res.results[cid]["olo"], np.float32)  # [NOSG,128,8,196]
        ohi = np.asarray(res.results[cid]["ohi"], np.float32)  # [NOSG,64,8,196]
        ob = out[cid * NB:(cid + 1) * NB]
        ob[:, 0:CLO] = olo.transpose(0, 2, 1, 3).reshape(NB, CLO, N)
        ob[:, CLO:C] = ohi.transpose(0, 2, 1, 3).reshape(NB, CHI, N)
    return out, res


def kernel(**inputs):
    out, _ = run(inputs, trace=False)
    return out


# revision 11
# speedup vs baseline: 1.1171x; 1.0699x over previous
"""Trainium2 Bass kernel for nn_CRA_46797963657479.

Math: the tail after cat_phi is linear in the gram matrix A, so it collapses:
    q[b]   = u3 + cat_phi[b] @ u4            (per-channel scalar, C)
    W[b,l] = q[b] . cat_phi[b][:,l] + K
    out[b] = xp[b] * W[b,:N] + yp[b] * W[b,N:]
with u3 = w3^T w5a, u4 = (w4[:, :2N]+w4[:, 2N:])^T w5b,
K = w5a.b3 + w5b.b4 + b5; BN folds into conv weights (W1',c1 / W2',c2).

Device pipeline per batch-pair (2 batches):
  PE   : 12 front matmuls (phi pre-act, bias via ones-row in x-hi chunk)
         + 4 back matmuls (q broadcast -> W on all partitions)
  DVE  : z = sum_l relu(ps)*u4 via scalar_tensor_tensor(max0, mult, accum)
         straight from PSUM (relu folded, no dependency on ACT),
         q = z + u3 (one tiny [128,3] add), y-side combine mult + adds
  ACT  : phi = relu(ps) -> SBUF bf16 (pure relu, no bias), 2 ops/pair
  GPS  : x-side combine mults (stt from PSUM), output DMA triggers (SWDGE)
  DMA  : bf16-only I/O; inputs on sync HWDGE queue, outputs on gpsimd queue

Channels 192 = 128 (lo) + 64 (hi); hi chunks of PSUM/phi are batch-packed
on 128 partitions (A on 0:64, B on 64:128). Output is written bf16 and
upcast to f32 on the host. Sharding: pure data parallel, 32 batches/core.
"""

import ml_dtypes
import numpy as np

import concourse.bass as bass
import concourse.bacc as bacc
import concourse.tile as tile
from concourse import mybir
from concourse.bass_utils import run_bass_kernel_spmd

F32 = mybir.dt.float32
BF16 = mybir.dt.bfloat16

B, N, C = 256, 196, 192
NCORES = 8
NB = B // NCORES          # 32 batches per core
NPAIR = NB // 2           # 16 pairs per core
L = 2 * N                 # 392 cat columns per batch
CLO, CHI = 128, C - 128   # 128 + 64 channel split
CHIA = CHI + 1            # hi input chunk + ones row (folds conv/BN bias)
SG = 2                    # pairs per input-DMA supergroup
OSG = 4                   # pairs per output-DMA supergroup
NSG = NPAIR // SG
NOSG = NPAIR // OSG

_CACHE = {}


def _build_program(adds_eng="gpsimd", reluhi_eng="scalar", out_eng="gpsimd", lag=2):
    nc = bacc.Bacc("TRN2", target_bir_lowering=False, debug=False)

    xlo_d = nc.dram_tensor("xlo", [NSG, CLO, SG * 2, 2, N], BF16, kind="ExternalInput")
    xhi_d = nc.dram_tensor("xhi", [NSG, CHIA, SG * 2, 2, N], BF16, kind="ExternalInput")
    wblob = nc.dram_tensor("wblob", [CLO, 768], BF16, kind="ExternalInput")
    cblob = nc.dram_tensor("cblob", [CLO, 398], F32, kind="ExternalInput")
    olo_d = nc.dram_tensor("olo", [NOSG, CLO, OSG * 2, N], BF16, kind="ExternalOutput")
    ohi_d = nc.dram_tensor("ohi", [NOSG, CHI, OSG * 2, N], BF16, kind="ExternalOutput")

    relu = mybir.ActivationFunctionType.Relu
    mult = mybir.AluOpType.mult
    add = mybir.AluOpType.add
    amax = mybir.AluOpType.max

    with tile.TileContext(nc) as tc:
        with (
            tc.tile_pool(name="consts", bufs=1) as consts,
            tc.tile_pool(name="xin", bufs=2) as xin,
            tc.tile_pool(name="phip", bufs=3) as phip,
            tc.tile_pool(name="junkp", bufs=2) as junkp,
            tc.tile_pool(name="zqp", bufs=3) as zqp,
            tc.tile_pool(name="workp", bufs=2) as workp,
            tc.tile_pool(name="outp", bufs=2) as outp,
            tc.tile_pool(name="pslo", bufs=2, space="PSUM") as pslo,
            tc.tile_pool(name="pshi", bufs=1, space="PSUM") as pshi,
            tc.tile_pool(name="psw", bufs=1, space="PSUM") as psw,
        ):
            wb = consts.tile([CLO, 768], BF16)
            nc.scalar.dma_start(out=wb[:], in_=wblob[:])
            cb = consts.tile([CLO, 398], F32)
            nc.scalar.dma_start(out=cb[:], in_=cblob[:])

            # weight tiles (lhsT = [cin, cout]); hi-contract tiles carry a
            # 65th row with the folded conv+BN bias
            twxa = wb[:, 0:128]
            twxb = wb[:, 128:192]
            twya = wb[:, 192:320]
            twyb = wb[:, 320:384]
            twxc = wb[0:CHIA, 384:512]
            twxd = wb[0:CHIA, 512:576]
            twyc = wb[0:CHIA, 576:704]
            twyd = wb[0:CHIA, 704:768]

            # u4 broadcast on all partitions f32 [128, 392] (+ 3 u3 cols)
            tu4f = cb[:, 0:392]
            tu3 = cb[:, 392:395]
            # [128, 2, 196] strided view of u4 (matches phi A/B slice shapes)
            tu4v = bass.AP(tensor=tu4f.tensor, offset=tu4f.offset,
                           ap=[tu4f.ap[0], [N, 2], [1, N]])

            pairs = []

            def emit_front(p, xl, xh, j):
                """j = pair index within its input supergroup."""
                xlo = xl[:, 2 * j:2 * j + 2, :, :]      # [128, 2b, 2s, 196]
                xhi = xh[:, 2 * j:2 * j + 2, :, :]      # [65, 2b, 2s, 196]

                ps_lo = pslo.tile([CLO, 2, 512], F32)   # banks: 0=x, 1=y
                ps_hi = pshi.tile([CLO, 512], F32)      # A on 0:64, B on 64:128

                nc.tensor.matmul(ps_lo[:, 0, 0:L], twxa, xlo[:, :, 0, :],
                                 start=True, stop=False)
                nc.tensor.matmul(ps_lo[:, 0, 0:L], twxc, xhi[:, :, 0, :],
                                 start=False, stop=True)
                nc.tensor.matmul(ps_lo[:, 1, 0:L], twya, xlo[:, :, 1, :],
                                 start=True, stop=False)
                nc.tensor.matmul(ps_lo[:, 1, 0:L], twyc, xhi[:, :, 1, :],
                                 start=False, stop=True)
                PB = (slice(0, 64), slice(64, 128))
                for s, cols in ((0, slice(0, N)), (1, slice(N, L))):
                    wl, wh = (twxb, twxd) if s == 0 else (twyb, twyd)
                    for b in (0, 1):
                        nc.tensor.matmul(ps_hi[PB[b], cols], wl,
                                         xlo[:, b, s, :], start=True, stop=False)
                        nc.tensor.matmul(ps_hi[PB[b], cols], wh,
                                         xhi[:, b, s, :], start=False, stop=True)

                # phi -> SBUF bf16 (pure relu; bias already in PSUM)
                phi_lo = phip.tile([CLO, 2, L], BF16, tag="phi_lo")
                phi_hi = phip.tile([CLO, L], BF16, tag="phi_hi")
                nc.scalar.activation(phi_lo[:], ps_lo[:, 0:2, 0:L], relu)
                if reluhi_eng == "gpsimd":
                    nc.gpsimd.tensor_scalar(phi_hi[:], ps_hi[:, 0:L],
                                            0.0, None, amax)
                else:
                    nc.scalar.activation(phi_hi[:], ps_hi[:, 0:L], relu)

                # z = sum_l relu(ps)*u4 straight from PSUM (relu folded)
                zf = zqp.tile([CLO, 3], F32, tag="zf")
                jA = junkp.tile([CLO, 2, N], BF16, tag="jA")
                jB = junkp.tile([CLO, 2, N], BF16, tag="jB")
                jh = junkp.tile([CLO, L], BF16, tag="jh")
                nc.vector.scalar_tensor_tensor(
                    out=jA[:], in0=ps_lo[:, 0:2, 0:N], scalar=0.0, in1=tu4v,
                    op0=amax, op1=mult, accum_out=zf[:, 0:1])
                nc.vector.scalar_tensor_tensor(
                    out=jB[:], in0=ps_lo[:, 0:2, N:L], scalar=0.0, in1=tu4v,
                    op0=amax, op1=mult, accum_out=zf[:, 1:2])
                nc.vector.scalar_tensor_tensor(
                    out=jh[:], in0=ps_hi[:, 0:L], scalar=0.0, in1=tu4f,
                    op0=amax, op1=mult, accum_out=zf[:, 2:3])

                # q = z + u3, bf16 (cols: A-lo, B-lo, hi batch-packed)
                qb = zqp.tile([CLO, 3], BF16, tag="qb")
                nc.vector.scalar_tensor_tensor(
                    out=qb[:], in0=zf[:], scalar=0.0, in1=tu3,
                    op0=mybir.AluOpType.bypass, op1=add)
                return dict(p=p, xlo=xlo, xhi=xhi,
                            phi_lo=phi_lo, phi_hi=phi_hi, qb=qb)

            def emit_back(st, og, jo):
                xlo, xhi = st["xlo"], st["xhi"]
                phi_lo, phi_hi, qb = st["phi_lo"], st["phi_hi"], st["qb"]

                ps_w = psw.tile([CLO, 2, 512], F32)     # banks: 0=W_A, 1=W_B
                nc.tensor.matmul(ps_w[:, 0, 0:L],
                                 qb[:, 0:1].broadcast_to([CLO, CLO]),
                                 phi_lo[:, 0:2, 0:N], start=True, stop=False)
                nc.tensor.matmul(ps_w[:, 0, 0:L],
                                 qb[0:64, 2:3].broadcast_to([64, CLO]),
                                 phi_hi[0:64, :], start=False, stop=True)
                nc.tensor.matmul(ps_w[:, 1, 0:L],
                                 qb[:, 1:2].broadcast_to([CLO, CLO]),
                                 phi_lo[:, 0:2, N:L], start=True, stop=False)
                nc.tensor.matmul(ps_w[:, 1, 0:L],
                                 qb[64:128, 2:3].broadcast_to([64, CLO]),
                                 phi_hi[64:128, :], start=False, stop=True)

                # combine: og = x*(Wx+K) + y*(Wy+K); ps_w banks = batch.
                # One stt per chunk: in0 is a [128, 2b, 2s, 196] view of ps_w
                # (bank stride for batch, N-offset for stream), in1 is the
                # matching x-input tile [*, 2b, 2s, 196].
                def wview(parts):
                    return bass.AP(
                        tensor=ps_w.tensor, offset=ps_w.offset,
                        ap=[[ps_w.ap[0][0], parts], [512, 2], [N, 2], [1, N]])

                t1 = workp.tile([CLO, 2, 2, N], BF16, tag="t1")
                t2 = workp.tile([CHI, 2, 2, N], BF16, tag="t2")
                nc.vector.scalar_tensor_tensor(
                    out=t1[:], in0=wview(CLO), scalar=KHOLD,
                    op0=add, op1=mult, in1=xlo[:, :, :, :])
                nc.vector.scalar_tensor_tensor(
                    out=t2[:], in0=wview(CHI), scalar=KHOLD,
                    op0=add, op1=mult, in1=xhi[0:CHI, :, :, :])
                b0 = 2 * jo
                ea = nc.gpsimd if adds_eng == "gpsimd" else nc.vector
                ea.tensor_tensor(og[:, 0, b0:b0 + 2, :], t1[:, :, 0, :],
                                 t1[:, :, 1, :], add)
                ea.tensor_tensor(og[0:CHI, 1, b0:b0 + 2, :], t2[:, :, 0, :],
                                 t2[:, :, 1, :], add)

            oeng = {"gpsimd": nc.gpsimd, "scalar": nc.scalar,
                    "sync": nc.sync}[out_eng]
            og = None
            for p in range(NPAIR):
                if p % SG == 0:
                    g = p // SG
                    xl = xin.tile([CLO, SG * 2, 2, N], BF16, tag="xl")
                    xh = xin.tile([CHIA, SG * 2, 2, N], BF16, tag="xh")
                    nc.sync.dma_start(out=xl[:], in_=xlo_d[g])
                    nc.sync.dma_start(out=xh[:], in_=xhi_d[g])
                if p % OSG == 0:
                    og = outp.tile([CLO, 2, OSG * 2, N], BF16, tag="og")
                st = emit_front(p, xl, xh, p % SG)
                pairs.append((st, og, p % OSG))
                if len(pairs) > lag:
                    sst, sog, sjo = pairs.pop(0)
                    emit_back(sst, sog, sjo)
                    if sjo == OSG - 1:
                        go = sst["p"] // OSG
                        oeng.dma_start(out=olo_d[go], in_=sog[:, 0, :, :])
                        oeng.dma_start(out=ohi_d[go], in_=sog[0:CHI, 1, :, :])
            while pairs:
                sst, sog, sjo = pairs.pop(0)
                emit_back(sst, sog, sjo)
                if sjo == OSG - 1:
                    go = sst["p"] // OSG
                    oeng.dma_start(out=olo_d[go], in_=sog[:, 0, :, :])
                    oeng.dma_start(out=ohi_d[go], in_=sog[0:CHI, 1, :, :])

    nc.compile()
    return nc


KHOLD = 0.0  # patched per-run before program build (K is a host constant)


def _host_prepack(d):
    """Fold BN, collapse the linear tail, build per-core constant arrays."""
    f = np.float32
    bf = ml_dtypes.bfloat16
    inv1 = d["g1"] / np.sqrt(d["v1"] + 1e-5)
    W1 = (d["w1"] * inv1[:, None]).astype(f)
    c1 = ((d["b1"] - d["m1"]) * inv1 + d["be1"]).astype(f)
    inv2 = d["g2"] / np.sqrt(d["v2"] + 1e-5)
    W2 = (d["w2"] * inv2[:, None]).astype(f)
    c2 = ((d["b2"] - d["m2"]) * inv2 + d["be2"]).astype(f)

    w4eff = d["w4"][:, :L] + d["w4"][:, L:]
    w5a, w5b = d["w5"][0, :C], d["w5"][0, C:]
    u3 = (w5a @ d["w3"]).astype(f)
    u4 = (w5b @ w4eff).astype(f)
    K = float(w5a @ d["b3"] + w5b @ d["b4"] + d["b5"][0])

    W1T = np.ascontiguousarray(W1.T)  # [cin, cout]
    W2T = np.ascontiguousarray(W2.T)

    wblob = np.zeros((CLO, 768), bf)

    def wput(c0, arr):
        a = np.asarray(arr, f)
        wblob[:a.shape[0], c0:c0 + a.shape[1]] = a.astype(bf)

    wput(0, W1T[:CLO, :CLO])
    wput(128, W1T[:CLO, CLO:C])
    wput(192, W2T[:CLO, :CLO])
    wput(320, W2T[:CLO, CLO:C])
    wput(384, np.concatenate([W1T[CLO:C, :CLO], c1[None, :CLO]], axis=0))
    wput(512, np.concatenate([W1T[CLO:C, CLO:C], c1[None, CLO:C]], axis=0))
    wput(576, np.concatenate([W2T[CLO:C, :CLO], c2[None, :CLO]], axis=0))
    wput(704, np.concatenate([W2T[CLO:C, CLO:C], c2[None, CLO:C]], axis=0))

    cblob = np.zeros((CLO, 398), f)
    cblob[:, 0:392] = np.broadcast_to(u4, (CLO, L))
    cblob[:, 392] = u3[:CLO]
    cblob[:, 393] = u3[:CLO]
    hi2 = np.concatenate([u3[CLO:C], u3[CLO:C]])
    cblob[:, 394] = hi2
    return {"wblob": wblob, "cblob": cblob}, K


def _pack_inputs(d):
    bf = ml_dtypes.bfloat16
    xt = np.ascontiguousarray(d["x"].transpose(0, 2, 1))  # [B, C, N]
    yt = np.ascontiguousarray(d["y"].transpose(0, 2, 1))
    xy = np.stack([xt, yt], axis=2).astype(bf)            # [B, C, 2, N]
    # [B, C, 2, N] -> per core [NSG, part, SG*2, 2, N]
    xlo = np.zeros((NCORES, NSG, CLO, SG * 2, 2, N), bf)
    xhi = np.zeros((NCORES, NSG, CHIA, SG * 2, 2, N), bf)
    v = xy.reshape(NCORES, NSG, SG * 2, C, 2, N)
    xlo[:] = v[:, :, :, 0:CLO].transpose(0, 1, 3, 2, 4, 5)
    xhi[:, :, 0:CHI] = v[:, :, :, CLO:C].transpose(0, 1, 3, 2, 4, 5)
    xhi[:, :, CHI] = 1.0
    return xlo, xhi


def run(inputs, trace=False):
    global KHOLD
    d = {k: np.asarray(v) for k, v in inputs.items()}
    consts, K = _host_prepack(d)
    KHOLD = K

    if "nc" not in _CACHE:
        _CACHE["nc"] = _build_program()
    nc = _CACHE["nc"]

    xlo, xhi = _pack_inputs(d)
    in_maps = []
    for cid in range(NCORES):
        m = dict(consts)
        m["xlo"] = np.ascontiguousarray(xlo[cid])
        m["xhi"] = np.ascontiguousarray(xhi[cid])
        in_maps.append(m)

    res = run_bass_kernel_spmd(nc, in_maps, list(range(NCORES)), trace=trace)

    out = np.empty((B, C, N), np.float32)
    for cid in range(NCORES):
        olo = np.asarray(res.results[cid]["olo"], np.float32)  # [NOSG,128,8,196]
        ohi = np.asarray(res.results[cid]["ohi"], np.float32)  # [NOSG,64,8,196]
        ob = out[cid * NB:(cid + 1) * NB]
        ob[:, 0:CLO] = olo.transpose(0, 2, 1, 3).reshape(NB, CLO, N)
        ob[:, CLO:C] = ohi.transpose(0, 2, 1, 3).reshape(NB, CHI, N)
    return out, res


def kernel(**inputs):
    out, _ = run(inputs, trace=False)
    return out


# revision 13
# speedup vs baseline: 1.2817x; 1.1473x over previous
"""Trainium2 Bass kernel for nn_CRA_46797963657479.

Math: the tail after cat_phi is linear in the gram matrix A, so it collapses:
    q[b]   = u3 + cat_phi[b] @ u4            (per-channel scalar, C)
    W[b,l] = q[b] . cat_phi[b][:,l] + K
    out[b] = xp[b] * W[b,:N] + yp[b] * W[b,N:]
with u3 = w3^T w5a, u4 = (w4[:, :2N]+w4[:, 2N:])^T w5b,
K = w5a.b3 + w5b.b4 + b5; BN folds into conv weights (W1',c1 / W2',c2).

Device pipeline per batch-pair (2 batches):
  PE   : 12 front matmuls (phi pre-act, bias via ones-row in x-hi chunk)
         + 4 back matmuls (q broadcast -> W on all partitions)
  DVE  : z = sum_l relu(ps)*u4 via scalar_tensor_tensor(max0, mult, accum)
         straight from PSUM (relu folded, no dependency on ACT),
         q = z + u3 (one tiny [128,3] add), y-side combine mult + adds
  ACT  : phi = relu(ps) -> SBUF bf16 (pure relu, no bias), 2 ops/pair
  GPS  : x-side combine mults (stt from PSUM), output DMA triggers (SWDGE)
  DMA  : bf16-only I/O; inputs on sync HWDGE queue, outputs on gpsimd queue

Channels 192 = 128 (lo) + 64 (hi); hi chunks of PSUM/phi are batch-packed
on 128 partitions (A on 0:64, B on 64:128). Output is written bf16 and
upcast to f32 on the host. Sharding: pure data parallel, 32 batches/core.
"""

import ml_dtypes
import numpy as np

import concourse.bass as bass
import concourse.bacc as bacc
import concourse.tile as tile
from concourse import mybir
from concourse.bass_utils import run_bass_kernel_spmd

F32 = mybir.dt.float32
BF16 = mybir.dt.bfloat16

B, N, C = 256, 196, 192
NCORES = 8
NB = B // NCORES          # 32 batches per core
NPAIR = NB // 2           # 16 pairs per core
L = 2 * N                 # 392 cat columns per batch
CLO, CHI = 128, C - 128   # 128 + 64 channel split
CHIA = CHI + 1            # hi input chunk + ones row (folds conv/BN bias)
SG = 2                    # pairs per input-DMA supergroup
OSG = 4                   # pairs per output-DMA supergroup
NSG = NPAIR // SG
NOSG = NPAIR // OSG

_CACHE = {}


def _build_program(adds_eng="gpsimd", reluhi_eng="scalar", out_eng="gpsimd", lag=3):
    nc = bacc.Bacc("TRN2", target_bir_lowering=False, debug=False)

    xlo_d = nc.dram_tensor("xlo", [NSG, CLO, SG * 2, 2, N], BF16, kind="ExternalInput")
    xhi_d = nc.dram_tensor("xhi", [NSG, CHIA, SG * 2, 2, N], BF16, kind="ExternalInput")
    wblob = nc.dram_tensor("wblob", [CLO, 768], BF16, kind="ExternalInput")
    cblob = nc.dram_tensor("cblob", [CLO, 398], F32, kind="ExternalInput")
    olo_d = nc.dram_tensor("olo", [NOSG, CLO, OSG * 2, N], BF16, kind="ExternalOutput")
    ohi_d = nc.dram_tensor("ohi", [NOSG, CHI, OSG * 2, N], BF16, kind="ExternalOutput")

    relu = mybir.ActivationFunctionType.Relu
    mult = mybir.AluOpType.mult
    add = mybir.AluOpType.add
    amax = mybir.AluOpType.max

    with tile.TileContext(nc) as tc:
        with (
            tc.tile_pool(name="consts", bufs=1) as consts,
            tc.tile_pool(name="xin", bufs=3) as xin,
            tc.tile_pool(name="phip", bufs=5) as phip,
            tc.tile_pool(name="junkp", bufs=3) as junkp,
            tc.tile_pool(name="zqp", bufs=5) as zqp,
            tc.tile_pool(name="workp", bufs=2) as workp,
            tc.tile_pool(name="outp", bufs=2) as outp,
            tc.tile_pool(name="pslo", bufs=2, space="PSUM") as pslo,
            tc.tile_pool(name="pshi", bufs=2, space="PSUM") as pshi,
            tc.tile_pool(name="psw", bufs=1, space="PSUM") as psw,
        ):
            wb = consts.tile([CLO, 768], BF16)
            nc.scalar.dma_start(out=wb[:], in_=wblob[:])
            cb = consts.tile([CLO, 398], F32)
            nc.scalar.dma_start(out=cb[:], in_=cblob[:])

            # weight tiles (lhsT = [cin, cout]); hi-contract tiles carry a
            # 65th row with the folded conv+BN bias
            twxa = wb[:, 0:128]
            twxb = wb[:, 128:192]
            twya = wb[:, 192:320]
            twyb = wb[:, 320:384]
            twxc = wb[0:CHIA, 384:512]
            twxd = wb[0:CHIA, 512:576]
            twyc = wb[0:CHIA, 576:704]
            twyd = wb[0:CHIA, 704:768]

            # u4 broadcast on all partitions f32 [128, 392] (+ 3 u3 cols)
            tu4f = cb[:, 0:392]
            tu3 = cb[:, 392:395]
            # [128, 2, 196] strided view of u4 (matches phi A/B slice shapes)
            tu4v = bass.AP(tensor=tu4f.tensor, offset=tu4f.offset,
                           ap=[tu4f.ap[0], [N, 2], [1, N]])

            pairs = []

            def emit_front(p, xl, xh, j):
                """j = pair index within its input supergroup."""
                xlo = xl[:, 2 * j:2 * j + 2, :, :]      # [128, 2b, 2s, 196]
                xhi = xh[:, 2 * j:2 * j + 2, :, :]      # [65, 2b, 2s, 196]

                ps_lo = pslo.tile([CLO, 2, 512], F32)   # banks: 0=x, 1=y
                ps_hi = pshi.tile([CLO, 512], F32)      # A on 0:64, B on 64:128

                nc.tensor.matmul(ps_lo[:, 0, 0:L], twxa, xlo[:, :, 0, :],
                                 start=True, stop=False)
                nc.tensor.matmul(ps_lo[:, 0, 0:L], twxc, xhi[:, :, 0, :],
                                 start=False, stop=True)
                nc.tensor.matmul(ps_lo[:, 1, 0:L], twya, xlo[:, :, 1, :],
                                 start=True, stop=False)
                nc.tensor.matmul(ps_lo[:, 1, 0:L], twyc, xhi[:, :, 1, :],
                                 start=False, stop=True)
                PB = (slice(0, 64), slice(64, 128))
                for s, cols in ((0, slice(0, N)), (1, slice(N, L))):
                    wl, wh = (twxb, twxd) if s == 0 else (twyb, twyd)
                    for b in (0, 1):
                        nc.tensor.matmul(ps_hi[PB[b], cols], wl,
                                         xlo[:, b, s, :], start=True, stop=False)
                        nc.tensor.matmul(ps_hi[PB[b], cols], wh,
                                         xhi[:, b, s, :], start=False, stop=True)

                # phi -> SBUF bf16 (pure relu; bias already in PSUM)
                phi_lo = phip.tile([CLO, 2, L], BF16, tag="phi_lo")
                phi_hi = phip.tile([CLO, L], BF16, tag="phi_hi")
                nc.scalar.activation(phi_lo[:], ps_lo[:, 0:2, 0:L], relu)
                if reluhi_eng == "gpsimd":
                    nc.gpsimd.tensor_scalar(phi_hi[:], ps_hi[:, 0:L],
                                            0.0, None, amax)
                else:
                    nc.scalar.activation(phi_hi[:], ps_hi[:, 0:L], relu)

                return dict(p=p, xlo=xlo, xhi=xhi, ps_lo=ps_lo,
                            ps_hi=ps_hi, phi_lo=phi_lo, phi_hi=phi_hi)

            def emit_z(st):
                # z = sum_l relu(ps)*u4 straight from PSUM (relu folded)
                ps_lo, ps_hi = st["ps_lo"], st["ps_hi"]
                zf = zqp.tile([CLO, 3], F32, tag="zf")
                jA = junkp.tile([CLO, 2, N], BF16, tag="jA")
                jB = junkp.tile([CLO, 2, N], BF16, tag="jB")
                jh = junkp.tile([CLO, L], BF16, tag="jh")
                nc.vector.scalar_tensor_tensor(
                    out=jA[:], in0=ps_lo[:, 0:2, 0:N], scalar=0.0, in1=tu4v,
                    op0=amax, op1=mult, accum_out=zf[:, 0:1])
                nc.vector.scalar_tensor_tensor(
                    out=jB[:], in0=ps_lo[:, 0:2, N:L], scalar=0.0, in1=tu4v,
                    op0=amax, op1=mult, accum_out=zf[:, 1:2])
                nc.vector.scalar_tensor_tensor(
                    out=jh[:], in0=ps_hi[:, 0:L], scalar=0.0, in1=tu4f,
                    op0=amax, op1=mult, accum_out=zf[:, 2:3])
                # q = z + u3, bf16 (cols: A-lo, B-lo, hi batch-packed)
                qb = zqp.tile([CLO, 3], BF16, tag="qb")
                nc.vector.scalar_tensor_tensor(
                    out=qb[:], in0=zf[:], scalar=0.0, in1=tu3,
                    op0=mybir.AluOpType.bypass, op1=add)
                st["qb"] = qb

            def emit_backmm(st):
                phi_lo, phi_hi, qb = st["phi_lo"], st["phi_hi"], st["qb"]
                ps_w = psw.tile([CLO, 2, 512], F32)     # banks: 0=W_A, 1=W_B
                nc.tensor.matmul(ps_w[:, 0, 0:L],
                                 qb[:, 0:1].broadcast_to([CLO, CLO]),
                                 phi_lo[:, 0:2, 0:N], start=True, stop=False)
                nc.tensor.matmul(ps_w[:, 0, 0:L],
                                 qb[0:64, 2:3].broadcast_to([64, CLO]),
                                 phi_hi[0:64, :], start=False, stop=True)
                nc.tensor.matmul(ps_w[:, 1, 0:L],
                                 qb[:, 1:2].broadcast_to([CLO, CLO]),
                                 phi_lo[:, 0:2, N:L], start=True, stop=False)
                nc.tensor.matmul(ps_w[:, 1, 0:L],
                                 qb[64:128, 2:3].broadcast_to([64, CLO]),
                                 phi_hi[64:128, :], start=False, stop=True)
                st["ps_w"] = ps_w

            def emit_combine(st, og, jo):
                xlo, xhi, ps_w = st["xlo"], st["xhi"], st["ps_w"]
                # combine: og = x*(Wx+K) + y*(Wy+K); ps_w banks = batch.
                # One stt per chunk: in0 is a [128, 2b, 2s, 196] view of ps_w
                # (bank stride for batch, N-offset for stream), in1 is the
                # matching x-input tile [*, 2b, 2s, 196].
                def wview(parts):
                    return bass.AP(
                        tensor=ps_w.tensor, offset=ps_w.offset,
                        ap=[[ps_w.ap[0][0], parts], [512, 2], [N, 2], [1, N]])

                t1 = workp.tile([CLO, 2, 2, N], BF16, tag="t1")
                t2 = workp.tile([CHI, 2, 2, N], BF16, tag="t2")
                nc.vector.scalar_tensor_tensor(
                    out=t1[:], in0=wview(CLO), scalar=KHOLD,
                    op0=add, op1=mult, in1=xlo[:, :, :, :])
                nc.vector.scalar_tensor_tensor(
                    out=t2[:], in0=wview(CHI), scalar=KHOLD,
                    op0=add, op1=mult, in1=xhi[0:CHI, :, :, :])
                b0 = 2 * jo
                ea = nc.gpsimd if adds_eng == "gpsimd" else nc.vector
                ea.tensor_tensor(og[:, 0, b0:b0 + 2, :], t1[:, :, 0, :],
                                 t1[:, :, 1, :], add)
                ea.tensor_tensor(og[0:CHI, 1, b0:b0 + 2, :], t2[:, :, 0, :],
                                 t2[:, :, 1, :], add)

            oeng = {"gpsimd": nc.gpsimd, "scalar": nc.scalar,
                    "sync": nc.sync}[out_eng]
            og = None
            for p in range(NPAIR + lag):
                if p < NPAIR and p % SG == 0:
                    g = p // SG
                    xl = xin.tile([CLO, SG * 2, 2, N], BF16, tag="xl")
                    xh = xin.tile([CHIA, SG * 2, 2, N], BF16, tag="xh")
                    if g == 0:  # split first load so pair 0 starts sooner
                        for h in range(SG):
                            b2 = slice(2 * h, 2 * h + 2)
                            nc.sync.dma_start(out=xl[:, b2], in_=xlo_d[g, :, b2])
                            nc.sync.dma_start(out=xh[:, b2], in_=xhi_d[g, :, b2])
                    else:
                        nc.sync.dma_start(out=xl[:], in_=xlo_d[g])
                        nc.sync.dma_start(out=xh[:], in_=xhi_d[g])
                if p < NPAIR and p % OSG == 0:
                    og = outp.tile([CLO, 2, OSG * 2, N], BF16, tag="og")
                ready = len(pairs) > lag - 1 or (p >= NPAIR and pairs)
                if ready:
                    # PE: back matmuls for pair p-lag (q long ready)
                    emit_backmm(pairs[0][0])
                if p < NPAIR:
                    st = emit_front(p, xl, xh, p % SG)
                if ready:
                    # DVE/GPS: combine for p-lag ahead of z(p) in the queues
                    sst, sog, sjo = pairs.pop(0)
                    emit_combine(sst, sog, sjo)
                    if sjo == OSG - 1:
                        go = sst["p"] // OSG
                        oeng.dma_start(out=olo_d[go], in_=sog[:, 0, :, :])
                        oeng.dma_start(out=ohi_d[go], in_=sog[0:CHI, 1, :, :])
                if p < NPAIR:
                    emit_z(st)
                    pairs.append((st, og, p % OSG))

    nc.compile()
    return nc


KHOLD = 0.0  # patched per-run before program build (K is a host constant)


def _host_prepack(d):
    """Fold BN, collapse the linear tail, build per-core constant arrays."""
    f = np.float32
    bf = ml_dtypes.bfloat16
    inv1 = d["g1"] / np.sqrt(d["v1"] + 1e-5)
    W1 = (d["w1"] * inv1[:, None]).astype(f)
    c1 = ((d["b1"] - d["m1"]) * inv1 + d["be1"]).astype(f)
    inv2 = d["g2"] / np.sqrt(d["v2"] + 1e-5)
    W2 = (d["w2"] * inv2[:, None]).astype(f)
    c2 = ((d["b2"] - d["m2"]) * inv2 + d["be2"]).astype(f)

    w4eff = d["w4"][:, :L] + d["w4"][:, L:]
    w5a, w5b = d["w5"][0, :C], d["w5"][0, C:]
    u3 = (w5a @ d["w3"]).astype(f)
    u4 = (w5b @ w4eff).astype(f)
    K = float(w5a @ d["b3"] + w5b @ d["b4"] + d["b5"][0])

    W1T = np.ascontiguousarray(W1.T)  # [cin, cout]
    W2T = np.ascontiguousarray(W2.T)

    wblob = np.zeros((CLO, 768), bf)

    def wput(c0, arr):
        a = np.asarray(arr, f)
        wblob[:a.shape[0], c0:c0 + a.shape[1]] = a.astype(bf)

    wput(0, W1T[:CLO, :CLO])
    wput(128, W1T[:CLO, CLO:C])
    wput(192, W2T[:CLO, :CLO])
    wput(320, W2T[:CLO, CLO:C])
    wput(384, np.concatenate([W1T[CLO:C, :CLO], c1[None, :CLO]], axis=0))
    wput(512, np.concatenate([W1T[CLO:C, CLO:C], c1[None, CLO:C]], axis=0))
    wput(576, np.concatenate([W2T[CLO:C, :CLO], c2[None, :CLO]], axis=0))
    wput(704, np.concatenate([W2T[CLO:C, CLO:C], c2[None, CLO:C]], axis=0))

    cblob = np.zeros((CLO, 398), f)
    cblob[:, 0:392] = np.broadcast_to(u4, (CLO, L))
    cblob[:, 392] = u3[:CLO]
    cblob[:, 393] = u3[:CLO]
    hi2 = np.concatenate([u3[CLO:C], u3[CLO:C]])
    cblob[:, 394] = hi2
    return {"wblob": wblob, "cblob": cblob}, K


def _pack_inputs(d):
    bf = ml_dtypes.bfloat16
    xt = np.ascontiguousarray(d["x"].transpose(0, 2, 1))  # [B, C, N]
    yt = np.ascontiguousarray(d["y"].transpose(0, 2, 1))
    xy = np.stack([xt, yt], axis=2).astype(bf)            # [B, C, 2, N]
    # [B, C, 2, N] -> per core [NSG, part, SG*2, 2, N]
    xlo = np.zeros((NCORES, NSG, CLO, SG * 2, 2, N), bf)
    xhi = np.zeros((NCORES, NSG, CHIA, SG * 2, 2, N), bf)
    v = xy.reshape(NCORES, NSG, SG * 2, C, 2, N)
    xlo[:] = v[:, :, :, 0:CLO].transpose(0, 1, 3, 2, 4, 5)
    xhi[:, :, 0:CHI] = v[:, :, :, CLO:C].transpose(0, 1, 3, 2, 4, 5)
    xhi[:, :, CHI] = 1.0
    return xlo, xhi


def run(inputs, trace=False):
    global KHOLD
    d = {k: np.asarray(v) for k, v in inputs.items()}
    consts, K = _host_prepack(d)
    KHOLD = K

    if "nc" not in _CACHE:
        _CACHE["nc"] = _build_program()
    nc = _CACHE["nc"]

    xlo, xhi = _pack_inputs(d)
    in_maps = []
    for cid in range(NCORES):
        m = dict(consts)
        m["xlo"] = np.ascontiguousarray(xlo[cid])
        m["xhi"] = np.ascontiguousarray(xhi[cid])
        in_maps.append(m)

    res = run_bass_kernel_spmd(nc, in_maps, list(range(NCORES)), trace=trace)

    out = np.empty((B, C, N), np.float32)
    for cid in range(NCORES):
        olo = np.asarray(res.results[cid]["olo"], np.float32)  # [NOSG,128,8,196]
        ohi = np.asarray(res.results[cid]["ohi"], np.float32)  # [NOSG,64,8,196]
        ob = out[cid * NB:(cid + 1) * NB]
        ob[:, 0:CLO] = olo.transpose(0, 2, 1, 3).reshape(NB, CLO, N)
        ob[:, CLO:C] = ohi.transpose(0, 2, 1, 3).reshape(NB, CHI, N)
    return out, res


def kernel(**inputs):
    out, _ = run(inputs, trace=False)
    return out
